# revision 1
# baseline (speedup 1.0000x reference)
"""Trainium2 Bass kernel for nn_CorrelationImage.

reference:
    corr_b = sum(map1[b] * map2[b])            # dot over C*H*W per sample
    corr   = corr / ||corr||_2                 # L2 norm over the batch
    out    = map1 + map2 * (1 - corr)[:, None, None, None]

Sharding: data-parallel over batch B=64 across 8 cores (8 samples/core).
Per core:
  1. stream the 8 (map1, map2) sample pairs into SBUF (kept resident);
     each sample's dot (DVE multiply + free-dim reduce) runs as soon as
     its 2 DMAs land, so the dot tail after the last load is ~5us,
  2. one ones(-1) matmul gives -c_i replicated on 128 partitions; the
     per-sample squares come from ONE ScalarE Square (same act table set
     as Sqrt, so no table reload); AllReduce-add of the 8 squared local
     dots (32B) gives the global sum of squares on every core,
  3. inv = 1/sqrt(ss) via ScalarE Sqrt(scale=-1) + DVE reciprocal, then
     s_i = 1 + (-c_i)*inv in one DVE tensor_scalar,
  4. out_i = map2_i * s_i + map1_i in place in the map2 buffer (ScalarE
     per-sample scale + DVE add), each sample's 1MB store streamed out
     immediately so stores overlap the remaining compute.

Notes from this hardware (axon-tunneled trn2, walrus path):
  - InstTensorTensorReduce and scalar_tensor_tensor (TensorScalarPtr on
    DVE) compile + pass CoreSim but HANG on this hardware; GpSimd cannot
    run TensorScalarPtr at all (verifier reject). Stick to tensor_mul /
    tensor_reduce / tensor_scalar / activation.
  - Act.Dsqrt has no activation table here; Abs_reciprocal_sqrt exists
    but is sim-unimplemented (worked around; unverified on HW).
  - Two concurrent collectives deadlock ncfw unless serialized by a data
    dependency. A serialized warmup AllReduce is net NEGATIVE (~+20us):
    ncfw re-dispatch after a prior collective costs ~30us, more than the
    ~11us cold-start it saves. The collective dispatch latency is the
    dominant noise term (10..55us observed run-to-run).
"""

import sys

import numpy as np

if "/opt/trn_rl_repo" not in sys.path:
    sys.path.insert(0, "/opt/trn_rl_repo")

B, C, H, W = 64, 64, 64, 64
N_CORES = 8
SPC = B // N_CORES  # samples per core
PART = 128
ELEMS = C * H * W  # 262144 per sample
FD = ELEMS // PART  # 2048 free-dim per sample tile

_cache = {}


def _build_nc(spc=SPC, fd=FD, n_cores=N_CORES, use_cc=True, cc_shared=True,
              warm_cc=False, abs_rsqrt=False):
    from contextlib import ExitStack

    from concourse import bacc, tile, mybir

    f32 = mybir.dt.float32
    Alu = mybir.AluOpType
    Act = mybir.ActivationFunctionType
    HALF = fd // 2

    nc = bacc.Bacc(
        "TRN2", target_bir_lowering=False, debug=False, num_devices=n_cores
    )
    m1d = nc.dram_tensor("map1", [spc, PART, fd], f32, kind="ExternalInput").ap()
    m2d = nc.dram_tensor("map2", [spc, PART, fd], f32, kind="ExternalInput").ap()
    outd = nc.dram_tensor("out", [spc, PART, fd], f32, kind="ExternalOutput").ap()

    with tile.TileContext(nc) as tc, ExitStack() as ctx:
        big = ctx.enter_context(tc.tile_pool(name="big", bufs=1))
        scv = ctx.enter_context(tc.tile_pool(name="scv", bufs=2))
        small = ctx.enter_context(tc.tile_pool(name="small", bufs=1))
        psum = ctx.enter_context(tc.tile_pool(name="psum", bufs=1, space="PSUM"))
        dram = ctx.enter_context(tc.tile_pool(name="dram", bufs=1, space="DRAM"))

        m1s = big.tile([PART, spc * fd], f32)
        m2s = big.tile([PART, spc * fd], f32)
        nones = small.tile([PART, PART], f32)
        nc.vector.memset(nones, -1.0)
        partials = small.tile([PART, spc], f32)
        # preload the act table off the critical path
        warm = small.tile([1, 1], f32)
        nc.vector.memset(warm, 1.0)
        nc.scalar.activation(
            out=warm,
            in_=warm,
            func=Act.Abs_reciprocal_sqrt if abs_rsqrt else Act.Sqrt,
        )

        # warmup collective: keeps the ncfw/TOPSP path hot so the real
        # AllReduce dispatches without the ~11us cold-start
        if use_cc and warm_cc:
            wa = small.tile([1, 1], f32)
            nc.vector.memset(wa, 0.0)
            warm_in = dram.tile([1], f32)
            warm_out = dram.tile(
                [1], f32, addr_space="Shared" if (cc_shared and n_cores > 4) else "Local"
            )
            nc.sync.dma_start(out=warm_in[:], in_=wa[:])
            nc.gpsimd.collective_compute(
                "AllReduce",
                Alu.add,
                replica_groups=[list(range(n_cores))],
                ins=[warm_in.opt()],
                outs=[warm_out.opt()],
            )

        # loads in sample order (contiguous 1MB DMAs); each sample's dot
        # (DVE multiply + DVE free-dim reduce) runs as soon as it lands,
        # tracking the loads at per-sample granularity
        for i in range(spc):
            sl = slice(i * fd, (i + 1) * fd)
            nc.sync.dma_start(out=m1s[:, sl], in_=m1d[i])
            nc.sync.dma_start(out=m2s[:, sl], in_=m2d[i])
            dv = scv.tile([PART, fd], f32, name="dv")
            nc.vector.tensor_mul(out=dv, in0=m1s[:, sl], in1=m2s[:, sl])
            nc.vector.tensor_reduce(
                out=partials[:, i : i + 1],
                in_=dv,
                axis=mybir.AxisListType.X,
                op=Alu.add,
            )

        # partition reduce of all dots at once: c8neg = -c_i, replicated
        c8neg = psum.tile([PART, spc], f32)
        nc.tensor.matmul(c8neg, nones, partials, start=True, stop=True)

        # per-sample squares in ONE ScalarE op (Square lives in the same
        # act table set as Abs_reciprocal_sqrt so there is no table reload)
        ssqo = small.tile([PART, spc], f32)
        nc.scalar.activation(out=ssqo, in_=c8neg, func=Act.Square)

        # AllReduce the 8 squared local dots (32B, same payload the
        # known-good baseline used)
        cc_in = dram.tile([spc], f32)
        nc.sync.dma_start(out=cc_in[:], in_=ssqo[0:1, :])
        sqsum = small.tile([1, spc], f32)
        if use_cc:
            cc_out = dram.tile(
                [spc],
                f32,
                addr_space="Shared" if (cc_shared and n_cores > 4) else "Local",
            )
            nc.gpsimd.collective_compute(
                "AllReduce",
                Alu.add,
                replica_groups=[list(range(n_cores))],
                ins=[cc_in.opt()],
                outs=[cc_out.opt()],
            )
            nc.sync.dma_start(out=sqsum[:], in_=cc_out[:])
        else:
            # debug only: pretend every core holds the same 8 samples
            nc.vector.tensor_scalar_mul(
                out=sqsum, in0=ssqo[0:1, :], scalar1=float(n_cores)
            )
        ss1 = small.tile([1, 1], f32)
        nc.vector.tensor_reduce(
            out=ss1, in_=sqsum, axis=mybir.AxisListType.X, op=Alu.add
        )

        # replicate -ss across partitions (K=1 matmul with the -1 tile),
        # then inv = 1/sqrt(ss)
        ssp = psum.tile([PART, 1], f32)
        nc.tensor.matmul(ssp, nones[0:1, :], ss1, start=True, stop=True)
        inv = small.tile([PART, 1], f32)
        if abs_rsqrt:
            # ONE ScalarE op; the abs eats the -ss sign
            nc.scalar.activation(out=inv, in_=ssp, func=Act.Abs_reciprocal_sqrt)
        else:
            # baseline-proven path: sqrt on ScalarE (scale=-1 flips the
            # replicated -ss back to +ss), reciprocal on DVE
            normb = small.tile([PART, 1], f32)
            nc.scalar.activation(out=normb, in_=ssp, func=Act.Sqrt, scale=-1.0)
            nc.vector.reciprocal(out=inv, in_=normb)
        s8 = small.tile([PART, spc], f32)
        nc.vector.tensor_scalar(
            out=s8,
            in0=c8neg,
            scalar1=inv,
            scalar2=1.0,
            op0=Alu.mult,
            op1=Alu.add,
        )

        # out_i = map2_i * s_i + map1_i: ScalarE scale + DVE add per
        # sample (baseline-proven ops), store streams out per sample
        for i in range(spc):
            sl = slice(i * fd, (i + 1) * fd)
            nc.scalar.activation(
                out=m2s[:, sl],
                in_=m2s[:, sl],
                func=Act.Copy,
                scale=s8[:, i : i + 1],
            )
            nc.vector.tensor_add(out=m2s[:, sl], in0=m2s[:, sl], in1=m1s[:, sl])
            nc.sync.dma_start(out=outd[i], in_=m2s[:, sl])

    nc.compile()
    return nc


def _get_nc():
    if "nc" not in _cache:
        _cache["nc"] = _build_nc()
    return _cache["nc"]


def kernel(map1, map2):
    from concourse.bass_utils import run_bass_kernel_spmd

    nc = _get_nc()
    m1 = np.ascontiguousarray(np.asarray(map1, dtype=np.float32)).reshape(
        N_CORES, SPC, PART, FD
    )
    m2 = np.ascontiguousarray(np.asarray(map2, dtype=np.float32)).reshape(
        N_CORES, SPC, PART, FD
    )
    in_maps = [{"map1": m1[c], "map2": m2[c]} for c in range(N_CORES)]
    res = run_bass_kernel_spmd(nc, in_maps, list(range(N_CORES)))
    out = np.concatenate(
        [res.results[c]["out"].reshape(SPC, C, H, W) for c in range(N_CORES)],
        axis=0,
    )
    return out

